# revision 8
# baseline (speedup 1.0000x reference)
"""Sharded kNN (ArgDistanceMeasure) on 8 TRN2 NeuronCores.

Architecture (v3): exact full-precision-ranking screen with the per-column
bias folded into the GEMM as two extra K rows; lean parallel consumers;
host does a shallow top-T candidate select + exact fp32 re-rank.

  - b ([65536, 512]) sharded row-wise across 8 cores (8192 cols each);
    a ([2048, 512]) replicated.
  - Ranking identity: argmin_j ||a_i - b_j + eps||_2 over j only needs
    t_j = 2*a_i.b_j - c_j (maximized), c_j = ||b_j||^2 - 2*eps*sum(b_j).
  - Device computes screen[i,j] = sum_{d<510} a8[i,d]*b8[j,d] - c'_j/2
    (= t_j/2 + tiny noise: fp8 input quant + 2 dropped dims), via 2 DoubleRow
    fp8 matmuls per [128 q x 512 col] block: K rows 0..509 are a/b dims,
    rows 510/511 are bias rows (a side: 8.0, 1.0; b side: -round(c'/16),
    -(c'-16*round(c'/16))/2), c' = c - mean(c).  PSUM holds bias-adjusted
    scores directly -> no host-side column permutation or bias bookkeeping.
  - Per [128 q x 2048 col] chunk (8 matmuls, ~1728ns warm, PE-bound):
      ACT: copy PSUM[:, 0:768] -> fp8 raw stage            (~900ns)
      DVE: reduce_max PSUM[:, 768:2048] in groups of 8 -> 160 fp8 maxima
           (~1490ns)
    Consumers write SEPARATE tiles (same-tile WAW serializes ACT->DVE, HW
    verified) and run in parallel under the PE budget, so the stream is the
    pure matmul floor: 512 DR matmuls x 216ns = 110.6us.
  - Out per core: raw 768/chunk (2-chunk-packed DMAs) + grouped 160/chunk
    (packed 4 m-tiles per DMA) = 6.6MB.
  - Host: screens are exact up to ~2.5 units of noise vs top-16 signal at
    z=3.5-4.3 (sigma 27.7); top-T=64 entries per query (group entries expand
    x8, ~500 candidates), exact fp32 re-rank with the reference's tie-break
    and buggy index bookkeeping.
"""
import numpy as np

NA, D, NB = 2048, 512, 65536
NCORES = 8
NB_SHARD = NB // NCORES  # 8192
CHUNK = 2048             # chunk width (4 PSUM banks)
RAW_W = 1024             # cols/chunk shipped raw (ACT); rest grouped (DVE)
G = 8                    # DVE max-group width
NGRP = (CHUNK - RAW_W) // G  # 160
TOPT = 192               # host: screen entries kept per query
NDUMMY = 7               # PE warmup matmuls on memset scratch (HAM clock flip)
EPS = 1e-6
M0 = 4                   # m-tiles covered by the first a m-slice
GPACK = 4                # m-tiles per grouped-out DMA


def build_kernel(na=NA, nb_shard=NB_SHARD, chunk=CHUNK):
    import concourse.mybir as mybir
    from concourse import bacc
    from concourse.tile import TileContext, add_dep_helper

    FP8 = mybir.dt.float8e4
    F32 = mybir.dt.float32
    DR = mybir.MatmulPerfMode.DoubleRow
    AX = mybir.AxisListType.X

    nseg = nb_shard // chunk  # 4
    kp_n = 2                  # two DoubleRow K-pairs (K=256 each)
    mt = na // 128            # 16

    nc = bacc.Bacc()

    # DoubleRow operands are [128, 2, cols] (two K-slices packed per
    # partition).  a is split into a first m-slice (queries 0..511) and the
    # rest; b chunk 0 is split into halves so the first matmuls' operands are
    # small, early DMAs.
    atm0_p = nc.declare_dram_parameter(
        "atm0", [128, 2 * 2 * 128 * M0], FP8, isOutput=False
    )
    atr_p = [
        nc.declare_dram_parameter(
            f"atrp{kp}", [128, 2 * 128 * (mt - M0)], FP8, isOutput=False
        )
        for kp in range(kp_n)
    ]
    bt0h_p = [
        [
            nc.declare_dram_parameter(
                f"bt0p{kp}{h}", [128, 2 * (chunk // 2)], FP8, isOutput=False
            )
            for h in ("a", "b")
        ]
        for kp in range(kp_n)
    ]
    bt_p = {
        (g, kp): nc.declare_dram_parameter(
            f"bt{g}p{kp}", [128, 2 * chunk], FP8, isOutput=False
        )
        for g in range(1, nseg)
        for kp in range(kp_n)
    }
    # raw screen out, TWO m-chunks packed per 128-row block:
    # row (m//2)*128 + r, col s*2*RAW_W + (m%2)*RAW_W + e
    raw_p = nc.declare_dram_parameter(
        "raw_s", [(na // 256) * 128, nseg * 2 * RAW_W], FP8, isOutput=True
    )
    # grouped-max out: row r, col s*mt*NGRP + m*NGRP + g  for query m*128+r
    grp_p = nc.declare_dram_parameter(
        "grp_s", [128, nseg * mt * NGRP], FP8, isOutput=True
    )

    with TileContext(nc) as tc:
        with (
            tc.tile_pool(name="weights", bufs=1) as wpool,
            tc.tile_pool(name="psum", bufs=2, space="PSUM") as ppool,
            tc.tile_pool(name="raws", bufs=4) as rpool,
            tc.tile_pool(name="grps", bufs=2) as gpool,
        ):
            # --- PE warmup: dummy DoubleRow matmuls on memset scratch ---
            wscr = wpool.tile([128, 2 * 512], FP8, tag="wscr", name="wscr")
            nc.gpsimd.memset(wscr, 0)
            w3 = wscr.rearrange("p (i c) -> p i c", i=2)
            # chunk PSUM is split into two 2-bank tiles with exactly ONE
            # consumer each (ACT<-psa raw copy, DVE<-psb grouped max): the
            # Tile framework serializes multiple readers of one tile (HW
            # verified +540ns/chunk), one reader per tile keeps ACT/DVE
            # parallel and the stream at the pure PE floor.
            ps_first = ppool.tile([128, RAW_W], F32, tag="psa", name="psa0")
            for _ in range(NDUMMY):
                nc.tensor.matmul(
                    ps_first[:, 0:512], w3[:, :, :128], w3,
                    start=True, stop=True, perf_mode=DR,
                )

            # --- critical first wave: a m-slice 0, b chunk 0 halves ---
            bt0h = [[None, None] for _ in range(kp_n)]
            half = chunk // 2
            for kp in range(kp_n):
                for hh in range(2):
                    bt0h[kp][hh] = wpool.tile(
                        [128, 2 * half], FP8, tag=f"bt0p{kp}h{hh}", name=f"bt0p{kp}h{hh}"
                    )
            nc.sync.dma_start(out=bt0h[0][0], in_=bt0h_p[0][0][:, :])
            atm0t = wpool.tile(
                [128, 2 * 2 * 128 * M0], FP8, tag="atm0", name="atm0t"
            )
            nc.sync.dma_start(out=atm0t, in_=atm0_p[:, :])
            atm0 = [atm0t[:, kp * 2 * 128 * M0 : (kp + 1) * 2 * 128 * M0]
                    for kp in range(kp_n)]
            nc.sync.dma_start(out=bt0h[1][0], in_=bt0h_p[1][0][:, :])
            for kp in range(kp_n):
                crit_dma = nc.sync.dma_start(out=bt0h[kp][1], in_=bt0h_p[kp][1][:, :])
            bt0h3 = [
                [t.rearrange("p (i c) -> p i c", i=2) for t in row] for row in bt0h
            ]
            # --- gated preloads: rest of a, b chunks 1..3 ---
            atr = []
            for kp in range(kp_n):
                t = wpool.tile(
                    [128, 2 * 128 * (mt - M0)], FP8, tag=f"atrp{kp}", name=f"atrp{kp}"
                )
                d = nc.sync.dma_start(out=t, in_=atr_p[kp][:, :])
                add_dep_helper(d.ins, crit_dma.ins, True, "preload priority")
                atr.append(t)
            bt_t = {}
            wave1 = None
            for g in range(1, nseg):
                for kp in range(kp_n):
                    t = wpool.tile(
                        [128, 2 * chunk], FP8, tag=f"bt{g}p{kp}", name=f"bt{g}p{kp}"
                    )
                    gate = crit_dma if g == 1 else wave1
                    d = nc.sync.dma_start(out=t, in_=bt_p[(g, kp)][:, :])
                    add_dep_helper(d.ins, gate.ins, True, "preload priority")
                    if g == 1:
                        wave1 = d
                    bt_t[(g, kp)] = t

            atm0_3 = [t.rearrange("p (i c) -> p i c", i=2) for t in atm0]
            atr_3 = [t.rearrange("p (i c) -> p i c", i=2) for t in atr]
            bt3 = {
                (g, kp): bt_t[(g, kp)].rearrange("p (i c) -> p i c", i=2)
                for g in range(1, nseg)
                for kp in range(kp_n)
            }

            for s in range(nseg):
                gt = gpool.tile([128, GPACK * NGRP], FP8, tag="gt", name=f"gt{s}")
                for m in range(mt):
                    if s == 0 and m == 0:
                        psa = ps_first
                    else:
                        psa = ppool.tile([128, RAW_W], F32, tag="psa", name=f"psa{s}_{m}")
                    psb = ppool.tile(
                        [128, chunk - RAW_W], F32, tag="psb", name=f"psb{s}_{m}"
                    )
                    for j in range(4):
                        ps = psa if j < 2 else psb
                        jc = j if j < 2 else j - 2
                        for kp in range(kp_n):
                            if m < M0:
                                lhsT3 = atm0_3[kp][:, :, m * 128 : (m + 1) * 128]
                            else:
                                lhsT3 = atr_3[kp][:, :, (m - M0) * 128 : (m - M0 + 1) * 128]
                            if s == 0:
                                h = j // 2
                                rhs3 = bt0h3[kp][h][:, :, (j % 2) * 512 : (j % 2 + 1) * 512]
                            else:
                                rhs3 = bt3[(s, kp)][:, :, j * 512 : (j + 1) * 512]
                            nc.tensor.matmul(
                                ps[:, jc * 512 : (jc + 1) * 512],
                                lhsT3,
                                rhs3,
                                start=(kp == 0),
                                stop=(kp == kp_n - 1),
                                perf_mode=DR,
                            )
                    if m % 2 == 0:
                        st = rpool.tile([128, 2 * RAW_W], FP8, tag="st", name="st")
                    off = (m % 2) * RAW_W
                    nc.scalar.copy(out=st[:, off : off + RAW_W], in_=psa)
                    ps3 = psb.rearrange("p (g w) -> p g w", w=G)
                    mg = m % GPACK
                    nc.vector.reduce_max(
                        out=gt[:, mg * NGRP : (mg + 1) * NGRP],
                        in_=ps3,
                        axis=AX,
                    )
                    mp_ = m // 2
                    last_pair = s == nseg - 1 and m >= mt - 2
                    if last_pair:
                        # split the final raw DMA so the first half departs
                        # one chunk earlier -> shorter drain tail
                        nc.sync.dma_start(
                            out=raw_p[
                                mp_ * 128 : (mp_ + 1) * 128,
                                s * 2 * RAW_W + off : s * 2 * RAW_W + off + RAW_W,
                            ],
                            in_=st[:, off : off + RAW_W],
                        )
                    elif m % 2 == 1:
                        nc.sync.dma_start(
                            out=raw_p[
                                mp_ * 128 : (mp_ + 1) * 128,
                                s * 2 * RAW_W : (s + 1) * 2 * RAW_W,
                            ],
                            in_=st,
                        )
                    if m % GPACK == GPACK - 1:
                        nc.sync.dma_start(
                            out=grp_p[
                                :,
                                s * mt * NGRP + (m - GPACK + 1) * NGRP
                                : s * mt * NGRP + (m + 1) * NGRP,
                            ],
                            in_=gt,
                        )
                        if m < mt - 1:
                            gt = gpool.tile(
                                [128, GPACK * NGRP], FP8, tag="gt", name=f"gt{s}_{m}"
                            )
    nc.compile()
    return nc


def _pack_dr(mat):
    """[256, cols] K-major -> [128, 2*cols] DoubleRow layout (slot 0 = K rows
    0..127, slot 1 = K rows 128..255)."""
    return np.ascontiguousarray(np.concatenate([mat[:128], mat[128:]], axis=1))


def make_in_maps(a, b):
    """Pack per-core fp8 DR operands; bias rows (K 510/511) fold the exact
    per-column bias c' into the GEMM."""
    import ml_dtypes

    E4 = ml_dtypes.float8_e4m3

    b2 = np.sum(b * b, axis=1)
    sb = b.sum(axis=1)
    c = (b2 - np.float32(2.0 * EPS) * sb).astype(np.float32)
    cmean = np.float32(c.mean(dtype=np.float64))
    cp = c - cmean
    r = np.round(cp / 16.0).astype(np.float32)
    f = ((cp - 16.0 * r) / 2.0).astype(np.float32)

    A = np.zeros((512, NA), dtype=E4)
    A[:510] = a[:, :510].T.astype(E4)
    A[510] = np.float32(8.0)
    A[511] = np.float32(1.0)

    def kp_pack(mat, kp):
        return _pack_dr(mat[kp * 256 : (kp + 1) * 256])

    in_maps = []
    for core in range(NCORES):
        sl = slice(core * NB_SHARD, (core + 1) * NB_SHARD)
        B = np.zeros((512, NB_SHARD), dtype=E4)
        B[:510] = b[sl, :510].T.astype(E4)
        B[510] = (-r[sl]).astype(E4)
        B[511] = (-f[sl]).astype(E4)
        im = {}
        im["atm0"] = np.ascontiguousarray(
            np.concatenate(
                [kp_pack(A[:, : 128 * M0], kp) for kp in range(2)], axis=1
            )
        )
        for kp in range(2):
            im[f"atrp{kp}"] = kp_pack(A[:, 128 * M0 :], kp)
        half = CHUNK // 2
        for kp in range(2):
            Bp = kp_pack(B, kp)  # [128, 2*NB_SHARD]
            for h, lo in (("a", 0), ("b", half)):
                im[f"bt0p{kp}{h}"] = np.ascontiguousarray(
                    np.concatenate(
                        [Bp[:, lo : lo + half],
                         Bp[:, NB_SHARD + lo : NB_SHARD + lo + half]],
                        axis=1,
                    )
                )
            for g in range(1, NB_SHARD // CHUNK):
                lo = g * CHUNK
                im[f"bt{g}p{kp}"] = np.ascontiguousarray(
                    np.concatenate(
                        [Bp[:, lo : lo + CHUNK],
                         Bp[:, NB_SHARD + lo : NB_SHARD + lo + CHUNK]],
                        axis=1,
                    )
                )
        in_maps.append(im)
    return in_maps


def merge_results(a, b, n, b_batch_size, results):
    """Decode fp8 screens, per-query top-TOPT entries (group entries expand
    x8), exact fp32 re-rank with reference tie-break + buggy bookkeeping."""
    nseg = NB_SHARD // CHUNK
    mt = NA // 128
    na = a.shape[0]

    parts = []
    for core in range(NCORES):
        raw = (
            results[core]["raw_s"]
            .astype(np.float32)
            .reshape(na // 256, 128, nseg, 2, RAW_W)
            .transpose(0, 3, 1, 2, 4)
            .reshape(na, nseg, RAW_W)
        )
        grp = (
            results[core]["grp_s"]
            .astype(np.float32)
            .reshape(128, nseg, mt, NGRP)
            .transpose(2, 0, 1, 3)
            .reshape(na, nseg, NGRP)
        )
        parts.append(np.concatenate([raw, grp], axis=2))  # [na, nseg, OUTW]
    outw = RAW_W + NGRP
    allscreen = np.stack(parts, axis=1).reshape(na, NCORES * nseg * outw)

    base = (
        np.arange(NCORES)[:, None, None] * NB_SHARD
        + np.arange(nseg)[None, :, None] * CHUNK
    )
    raw_ids = (base + np.arange(RAW_W)[None, None, :]).reshape(-1)
    grp_base = (base + RAW_W + np.arange(NGRP)[None, None, :] * G).reshape(-1)
    nent = NCORES * nseg * outw
    is_raw_entry = np.zeros(nent, bool)
    ent_raw_id = np.zeros(nent, np.int64)
    ent_grp_id = np.zeros(nent, np.int64)
    ent = np.arange(nent).reshape(NCORES, nseg, outw)
    is_raw_entry[ent[:, :, :RAW_W].ravel()] = True
    ent_raw_id[ent[:, :, :RAW_W].ravel()] = raw_ids
    ent_grp_id[ent[:, :, RAW_W:].ravel()] = grp_base

    sel = np.argpartition(-allscreen, TOPT, axis=1)[:, :TOPT]

    a2 = np.sum(a * a, axis=1)
    sa = np.sum(a, axis=1)
    b2 = np.sum(b * b, axis=1)
    sb = np.sum(b, axis=1)
    eps = np.float32(EPS)
    out = np.empty((na, n), dtype=np.int64)
    BQ = 256
    for q0 in range(0, na, BQ):
        q1 = min(q0 + BQ, na)
        blk = sel[q0:q1]
        cands = []
        for qi in range(q1 - q0):
            e = blk[qi]
            er = e[is_raw_entry[e]]
            eg = e[~is_raw_entry[e]]
            gexp = (ent_grp_id[eg][:, None] + np.arange(G)[None, :]).ravel()
            cands.append(np.concatenate([ent_raw_id[er], gexp]))
        L = max(len(x) for x in cands)
        Cc = np.empty((q1 - q0, L), np.int64)
        npad = np.empty(q1 - q0, np.int64)
        for qi, x in enumerate(cands):
            Cc[qi, : len(x)] = x
            npad[qi] = len(x)
            if len(x) < L:
                Cc[qi, len(x) :] = x[0]
        A = a[q0:q1]
        Bc = b[Cc]
        cross = np.matmul(Bc, A[:, :, None])[..., 0].astype(np.float32)
        sq = (
            a2[q0:q1, None]
            + b2[Cc]
            - np.float32(2.0) * cross
            + np.float32(2.0) * eps * (sa[q0:q1, None] - sb[Cc])
            + np.float32(D) * eps * eps
        )
        dist = np.sqrt(np.maximum(sq, np.float32(0.0)))
        # padded slots duplicate x[0]; push them past any real candidate so a
        # duplicate can never displace a true member from the top-n
        dist[np.arange(L)[None, :] >= npad[:, None]] = np.float32(np.inf)
        ordr = np.lexsort((Cc, dist), axis=1)[:, :n]
        rows = np.arange(q1 - q0)[:, None]
        out[q0:q1] = Cc[rows, ordr]
    buggy = (out % b_batch_size) + (out // b_batch_size)
    return buggy.astype(np.int32)


def kernel(a, b, n, b_batch_size, trace=False):
    from concourse.bass_utils import run_bass_kernel_spmd

    a = np.ascontiguousarray(np.asarray(a, dtype=np.float32))
    b = np.ascontiguousarray(np.asarray(b, dtype=np.float32))
    n = int(n)
    b_batch_size = int(b_batch_size)

    nc = build_kernel()
    in_maps = make_in_maps(a, b)
    res = run_bass_kernel_spmd(
        nc, in_maps, core_ids=list(range(NCORES)), trace=trace
    )
    out = merge_results(a, b, n, b_batch_size, res.results)
    if trace:
        return out, res
    return out
